# revision 21
# baseline (speedup 1.0000x reference)
"""RAFT correlation-pyramid lookup kernel for 8 trn2 NeuronCores.

Shard: each core takes all 4 batches x 8 pixel rows (i-axis) of the
64x64 grid.  Per core:
  1. matmul (fp32r) f1^T @ [f2 | pooled-f2-pyramids]  -> corr volumes
  2. ACT drains PSUM -> SBUF fp16, DMA to a DRAM bounce tile
  3. element-granular indirect-DMA gathers per-pixel dynamic windows
  4. DVE reconstructs bilinear lookups as separable 3-tap MACs
Weights / gather indices are computed on-device from the flow input.
"""

import os
import sys

import numpy as np

sys.path.insert(0, "/opt/trn_rl_repo")

B = 4
C = 256
H8 = W8 = 64
RI = 8  # i-rows per core
NCORES = 8
NT = 16  # pixel tiles per core (128 pixels each)
NPIX = NT * 128

HK = [64, 32, 16, 8]  # level map sizes
SEG = [0, 4096, 5120, 5376]  # level offsets within a pixel's volume
VOLW = 5440  # per-pixel volume length (all 4 levels)
L0LEN = 651  # 10*64+11
L123LEN = 331  # 10*32+11  (levels 2,3 padded to this)
NL = 41  # diamond offsets
ROWLEN = [1, 3, 5, 7, 9, 7, 5, 3, 1]  # diamond row lengths (dy=-4..4)
ROWOFF = np.concatenate([[0], np.cumsum(ROWLEN)[:-1]]).tolist()

_CACHE = {}
_DBG = {}


def _consts(core):
    """Constant input tensors for one core (fp32)."""
    p = np.arange(128)
    t = np.arange(NT)
    k = np.arange(4)
    j = np.arange(9)
    hk = np.array(HK, dtype=np.float64)

    # i/j pixel coordinates:  t=(b,ihi), part=(i2,jcol)
    ihi = t % 4
    i2 = p // 64
    iconst = (8 * core + ihi[None, :] * 2 + i2[:, None]).astype(np.float32)
    jconst = (p % 64).astype(np.float32)[:, None]

    cinvs64 = np.broadcast_to((0.5**k)[:, None], (4, NT))
    chi64 = np.broadcast_to((hk + 5.2)[:, None], (4, NT))
    ct64 = np.broadcast_to(((hk - 1.0) / hk)[:, None], (4, NT))
    cw64 = np.broadcast_to(hk[:, None], (4, NT))
    segk = np.array(SEG, dtype=np.float64)
    cbase = p[:, None, None] * VOLW + VOLW + segk[None, :, None] - hk[None, :, None] - 1.0
    cbase64 = np.broadcast_to(cbase, (128, 4, NT))

    def rep64(a):  # [4,NT] -> [128, 64]
        return np.broadcast_to(a[None], (128, 4, NT)).reshape(128, 64).astype(np.float32)

    def rep576(a):  # [9,4] -> [128, 576]
        x = np.broadcast_to(a[:, :, None], (9, 4, NT))
        return np.broadcast_to(x[None], (128, 9, 4, NT)).reshape(128, 576).astype(np.float32)

    cdyt = rep576((j - 4)[:, None] * ((hk - 1.0) / hk)[None, :])
    ch05 = rep576(np.broadcast_to((hk - 0.5)[None, :], (9, 4)))
    ch15 = rep576(np.broadcast_to((hk - 1.5)[None, :], (9, 4)))
    cj = rep576(np.broadcast_to(j[:, None].astype(np.float64), (9, 4)))
    csk = rep576(np.broadcast_to((0.0625 * 0.25**k)[None, :], (9, 4)))

    return {
        "iconst": iconst,
        "jconst": jconst.astype(np.float32),
        "cinvs64": rep64(cinvs64),
        "chi64": rep64(chi64),
        "ct64": rep64(ct64),
        "cw64": rep64(cw64),
        "cbase64": cbase64.reshape(128, 64).astype(np.float32),
        "cdyt": cdyt,
        "ch05": ch05,
        "ch15": ch15,
        "cj": cj,
        "csk": csk,
    }


CSHAPES = {
    "iconst": [128, NT], "jconst": [128, 1],
    "cinvs64": [128, 64], "chi64": [128, 64], "ct64": [128, 64],
    "cw64": [128, 64], "cbase64": [128, 64],
    "cdyt": [128, 576], "ch05": [128, 576], "ch15": [128, 576],
    "cj": [128, 576], "csk": [128, 576],
}


def _build():
    import concourse.bass as bass
    import concourse.tile as tile
    from concourse import bacc, mybir

    f32 = mybir.dt.float32
    f32r = mybir.dt.float32r
    f16 = mybir.dt.float16
    i32 = mybir.dt.int32
    Alu = mybir.AluOpType

    nc = bacc.Bacc("TRN2", target_bir_lowering=False, debug=False, num_devices=NCORES)

    f1 = nc.dram_tensor("f1", [B, C, RI, W8], f32r, kind="ExternalInput")
    f2 = nc.dram_tensor("f2", [B, C, H8, W8], f32r, kind="ExternalInput")
    flow = nc.dram_tensor("flow", [B, 2, RI, W8], f32, kind="ExternalInput")
    cdram = {n: nc.dram_tensor(n, s, f32, kind="ExternalInput") for n, s in CSHAPES.items()}
    out = nc.dram_tensor("out", [NPIX, NL * 4], f32, kind="ExternalOutput")
    dbg_dump = os.environ.get("KDBG_DUMP", "0") == "1"
    if dbg_dump:
        dbg_idx = nc.dram_tensor("dbg_idx", [128, 64], i32, kind="ExternalOutput")
        dbg_w = nc.dram_tensor("dbg_w", [128, 6 * 576], f32, kind="ExternalOutput")
        dbg_vol = nc.dram_tensor("dbg_vol", [128, VOLW], f16, kind="ExternalOutput")
        dbg_f = nc.dram_tensor("dbg_f", [128, 1244], f16, kind="ExternalOutput")
        dbg_rect = nc.dram_tensor("dbg_rect", [128, 324], f32, kind="ExternalOutput")

    with tile.TileContext(nc) as tc:
        with (
            tc.tile_pool(name="const", bufs=1) as cp,
            tc.tile_pool(name="wts", bufs=1) as wp,
        ):
            ct = {}
            for n, s in CSHAPES.items():
                ct[n] = cp.tile(s, f32, tag=n, name=n)
                nc.sync.dma_start(out=ct[n][:], in_=cdram[n][:])

            # f1 lhsT tiles: [128c, 512pix] per (b, kc)
            f1t = {}
            for b in range(B):
                for kc in range(2):
                    f1t[b, kc] = cp.tile([128, 512], f32r, tag=f"f1_{b}_{kc}", name=f"f1t{b}{kc}")
                    src = f1[b, kc * 128:(kc + 1) * 128, :, :].rearrange("c i j -> c (i j)")
                    nc.sync.dma_start(out=f1t[b, kc][:], in_=src)

            # zero row for DRAM pad rows
            zrow = cp.tile([1, VOLW], f16, tag="zrow", name="zrow")
            nc.vector.memset(zrow[:], 0.0)

            # ---- flow -> lookup weights + gather indices (DVE) ----
            W = {}
            idx32 = wp.tile([128, 64], i32, tag="idx32", name="idx32")
            with tc.tile_pool(name="wscratch", bufs=1) as sp:
                fy = sp.tile([128, NT], f32, tag="fy", name="fy")
                fx = sp.tile([128, NT], f32, tag="fx", name="fx")
                fsrc = flow[:].rearrange("b c (ihi i2) j -> c (i2 j) b ihi", i2=2)
                for b in range(B):
                    nc.sync.dma_start(out=fy[:, b * 4:(b + 1) * 4], in_=fsrc[0, :, b])
                    nc.sync.dma_start(out=fx[:, b * 4:(b + 1) * 4], in_=fsrc[1, :, b])

                yc = sp.tile([128, NT], f32, tag="yc", name="yc")
                xc = sp.tile([128, NT], f32, tag="xc", name="xc")
                nc.vector.tensor_add(yc[:], fy[:], ct["iconst"][:])
                nc.vector.tensor_add(xc[:], fx[:], ct["jconst"][:].to_broadcast([128, NT]))

                Y0J0 = {}
                for ax, coord in (("y", yc), ("x", xc)):
                    ck = sp.tile([128, 64], f32, tag=f"ck_{ax}", name=f"ck{ax}")
                    nc.vector.tensor_tensor(
                        out=ck[:].rearrange("p (k t) -> p k t", k=4),
                        in0=coord[:].unsqueeze(1).to_broadcast([128, 4, NT]),
                        in1=ct["cinvs64"][:].rearrange("p (k t) -> p k t", k=4), op=Alu.mult)
                    nc.vector.tensor_tensor(out=ck[:], in0=ck[:], in1=ct["chi64"][:], op=Alu.min)
                    nc.vector.tensor_scalar_max(ck[:], ck[:], -5.2)
                    nc.vector.tensor_tensor(out=ck[:], in0=ck[:], in1=ct["ct64"][:], op=Alu.mult)

                    ybar = sp.tile([128, 576], f32, tag=f"ybar_{ax}", name=f"ybar{ax}")
                    nc.vector.tensor_tensor(
                        out=ybar[:].rearrange("p (j q) -> p j q", j=9),
                        in0=ck[:].unsqueeze(1).to_broadcast([128, 9, 64]),
                        in1=ct["cdyt"][:].rearrange("p (j q) -> p j q", j=9), op=Alu.add)
                    # exact floor: round-to-nearest via +2^23, then correct
                    frac = sp.tile([128, 576], f32, tag=f"frac_{ax}", name=f"frac{ax}")
                    y0f = sp.tile([128, 576], f32, tag=f"y0f_{ax}", name=f"y0f{ax}")
                    cmp = sp.tile([128, 576], f32, tag=f"cmp_{ax}", name=f"cmp{ax}")
                    nc.vector.tensor_scalar_add(y0f[:], ybar[:], 12582912.0)
                    nc.vector.tensor_scalar_add(y0f[:], y0f[:], -12582912.0)
                    nc.vector.tensor_tensor(out=cmp[:], in0=y0f[:], in1=ybar[:], op=Alu.is_gt)
                    nc.vector.tensor_sub(y0f[:], y0f[:], cmp[:])
                    nc.vector.tensor_sub(frac[:], ybar[:], y0f[:])
                    Y0J0[ax] = y0f

                    v0 = sp.tile([128, 576], f32, tag=f"v0_{ax}", name=f"v0{ax}")
                    v1 = sp.tile([128, 576], f32, tag=f"v1_{ax}", name=f"v1{ax}")
                    tmp = sp.tile([128, 576], f32, tag=f"tmp_{ax}", name=f"tmp{ax}")
                    # valid(y0):   y0 in [0, h-1]
                    nc.vector.tensor_scalar(v0[:], y0f[:], -0.1, None, op0=Alu.is_ge)
                    nc.vector.tensor_tensor(out=tmp[:], in0=y0f[:], in1=ct["ch05"][:], op=Alu.is_le)
                    nc.vector.tensor_mul(v0[:], v0[:], tmp[:])
                    # valid(y0+1): y0 in [-1, h-2]
                    nc.vector.tensor_scalar(v1[:], y0f[:], -1.1, None, op0=Alu.is_ge)
                    nc.vector.tensor_tensor(out=tmp[:], in0=y0f[:], in1=ct["ch15"][:], op=Alu.is_le)
                    nc.vector.tensor_mul(v1[:], v1[:], tmp[:])

                    w0 = sp.tile([128, 576], f32, tag=f"w0_{ax}", name=f"w0{ax}")
                    w1 = sp.tile([128, 576], f32, tag=f"w1_{ax}", name=f"w1{ax}")
                    nc.vector.tensor_mul(w1[:], frac[:], v1[:])
                    nc.vector.tensor_scalar(w0[:], frac[:], 1.0, -1.0, op0=Alu.subtract, op1=Alu.mult)
                    nc.vector.tensor_mul(w0[:], w0[:], v0[:])
                    if ax == "x":  # fold level scale (1/16 * 4^-k) into x weights
                        nc.vector.tensor_mul(w0[:], w0[:], ct["csk"][:])
                        nc.vector.tensor_mul(w1[:], w1[:], ct["csk"][:])

                    ey = sp.tile([128, 576], f32, tag=f"ey_{ax}", name=f"ey{ax}")
                    nc.vector.tensor_tensor(
                        out=ey[:].rearrange("p (j q) -> p j q", j=9),
                        in0=y0f[:, 0:64].unsqueeze(1).to_broadcast([128, 9, 64]),
                        in1=y0f[:].rearrange("p (j q) -> p j q", j=9), op=Alu.subtract)
                    nc.vector.tensor_add(ey[:], ey[:], ct["cj"][:])

                    wm1 = wp.tile([128, 576], f32, tag=f"wm1_{ax}", name=f"wm1{ax}")
                    w_0 = wp.tile([128, 576], f32, tag=f"w_0_{ax}", name=f"w_0{ax}")
                    wp1 = wp.tile([128, 576], f32, tag=f"wp1_{ax}", name=f"wp1{ax}")
                    tmp2 = sp.tile([128, 576], f32, tag=f"tmp2_{ax}", name=f"tmp2{ax}")
                    nc.vector.tensor_mul(wm1[:], w0[:], ey[:])
                    nc.vector.tensor_mul(tmp2[:], w1[:], ey[:])
                    nc.vector.tensor_sub(w_0[:], w0[:], wm1[:])
                    nc.vector.tensor_add(w_0[:], w_0[:], tmp2[:])
                    nc.vector.tensor_sub(wp1[:], w1[:], tmp2[:])
                    W[ax, -1], W[ax, 0], W[ax, 1] = wm1, w_0, wp1

                # gather indices: [128, 64] int32 (fp16-element offsets)
                idxf = sp.tile([128, 64], f32, tag="idxf", name="idxf")
                nc.vector.tensor_tensor(out=idxf[:], in0=Y0J0["y"][:, 0:64], in1=ct["cw64"][:], op=Alu.mult)
                nc.vector.tensor_add(idxf[:], idxf[:], Y0J0["x"][:, 0:64])
                nc.vector.tensor_add(idxf[:], idxf[:], ct["cbase64"][:])
                # store as [t, k] so per-tile index slices are contiguous
                nc.vector.tensor_copy(
                    out=idx32[:].rearrange("p (t k) -> p t k", k=4).transpose([0, 2, 1]),
                    in_=idxf[:].rearrange("p (k t) -> p k t", t=NT))

            if dbg_dump:
                nc.sync.dma_start(out=dbg_idx[:], in_=idx32[:])
                for qi, key in enumerate((("x", -1), ("x", 0), ("x", 1), ("y", -1), ("y", 0), ("y", 1))):
                    nc.sync.dma_start(out=dbg_w[:, qi * 576:(qi + 1) * 576], in_=W[key][:])

            # matmul N-chunk plan: (source, src_off, width, vol_off)
            nplan = [("f2", n * 512, 512, n * 512) for n in range(8)] + [
                ("pyr", 0, 512, 4096), ("pyr", 512, 512, 4608),
                ("pyr", 1024, 256, 5120), ("pyr", 1280, 64, 5376)]

            with (
                tc.tile_pool(name="main", bufs=2) as mp,
                tc.tile_pool(name="fio", bufs=3) as fp,
                tc.tile_pool(name="dram", bufs=3, space="DRAM") as dp,
                tc.tile_pool(name="psum", bufs=8, space="PSUM") as pp,
            ):
                nb = int(os.environ.get("KDBG_NB", str(B)))
                nm_ = int(os.environ.get("KDBG_NM", "4"))
                for b in range(nb):
                    f2t = {}
                    pyrt = {}
                    for kc in range(2):
                        f2t[kc] = mp.tile([128, 4096], f32r, tag=f"f2_{kc}", name=f"f2t{kc}")
                        src = f2[b, kc * 128:(kc + 1) * 128, :, :].rearrange("c u v -> c (u v)")
                        nc.sync.dma_start(out=f2t[kc][:], in_=src)
                        # pooled pyramids (pure sums; scales folded into x-weights)
                        pyrt[kc] = mp.tile([128, 1344], f32r, tag=f"pyr_{kc}", name=f"pyrt{kc}")
                        A = mp.tile([128, 2048], f32r, tag="poolA", name="poolA")
                        s3 = f2t[kc][:].rearrange("p (x two) -> p x two", two=2)
                        nc.vector.tensor_tensor(out=A[:], in0=s3[:, :, 0], in1=s3[:, :, 1], op=Alu.add)
                        a3 = A[:].rearrange("p (uh two v) -> p uh two v", two=2, v=32)
                        nc.vector.tensor_tensor(
                            out=pyrt[kc][:, 0:1024].rearrange("p (u v) -> p u v", v=32),
                            in0=a3[:, :, 0, :], in1=a3[:, :, 1, :], op=Alu.add)
                        s3 = pyrt[kc][:, 0:1024].rearrange("p (x two) -> p x two", two=2)
                        nc.vector.tensor_tensor(out=A[:, 0:512], in0=s3[:, :, 0], in1=s3[:, :, 1], op=Alu.add)
                        a3 = A[:, 0:512].rearrange("p (uh two v) -> p uh two v", two=2, v=16)
                        nc.vector.tensor_tensor(
                            out=pyrt[kc][:, 1024:1280].rearrange("p (u v) -> p u v", v=16),
                            in0=a3[:, :, 0, :], in1=a3[:, :, 1, :], op=Alu.add)
                        s3 = pyrt[kc][:, 1024:1280].rearrange("p (x two) -> p x two", two=2)
                        nc.vector.tensor_tensor(out=A[:, 0:128], in0=s3[:, :, 0], in1=s3[:, :, 1], op=Alu.add)
                        a3 = A[:, 0:128].rearrange("p (uh two v) -> p uh two v", two=2, v=8)
                        nc.vector.tensor_tensor(
                            out=pyrt[kc][:, 1280:1344].rearrange("p (u v) -> p u v", v=8),
                            in0=a3[:, :, 0, :], in1=a3[:, :, 1, :], op=Alu.add)

                    for m in range(nm_):
                        t = b * 4 + m
                        volsb = fp.tile([128, VOLW], f16, tag="volsb", name="volsb", bufs=2)
                        for (srcname, soff, width, doff) in nplan:
                            ps = pp.tile([128, width], f32, tag="ps", name="ps")
                            rhs_t = f2t if srcname == "f2" else pyrt
                            for kc in range(2):
                                nc.tensor.matmul(
                                    out=ps[:],
                                    lhsT=f1t[b, kc][:, m * 128:(m + 1) * 128],
                                    rhs=rhs_t[kc][:, soff:soff + width],
                                    start=(kc == 0), stop=(kc == 1))
                            nc.scalar.copy(out=volsb[:, doff:doff + width], in_=ps[:])

                        vols = dp.tile([130, VOLW], f16, tag="vols", name="vols",
                                       bufs=int(os.environ.get("KDBG_VOLBUFS", "0")) or None)
                        _DBG.setdefault("vols", []).append(vols.tensor.name)
                        nc.sync.dma_start(out=vols[0:1, :], in_=zrow[:])
                        nc.sync.dma_start(out=vols[129:130, :], in_=zrow[:])
                        nc.sync.dma_start(out=vols[1:129, :], in_=volsb[:])

                        vflat = vols[:].rearrange("a b -> (a b)").unsqueeze(1)
                        Fk = []
                        for k in range(4):
                            flen = 10 * HK[k] + 11
                            ftile = fp.tile([128, flen], f16, tag=f"F{k}", name=f"F{k}")
                            nc.gpsimd.indirect_dma_start(
                                out=ftile[:], out_offset=None, in_=vflat,
                                in_offset=bass.IndirectOffsetOnAxis(
                                    ap=idx32[:, t * 4 + k:t * 4 + k + 1], axis=0))
                            Fk.append(ftile)
                        _DBG.setdefault("F0", []).append(Fk[0].tensor.name)
                        if dbg_dump and t == 0:
                            nc.sync.dma_start(out=dbg_vol[:], in_=volsb[:])
                            fo = 0
                            for k in range(4):
                                fl = 10 * HK[k] + 11
                                nc.sync.dma_start(out=dbg_f[:, fo:fo + fl], in_=Fk[k][:])
                                fo += fl

                        # ---- separable 3-tap reconstruction ----
                        rect4 = fp.tile([128, 324], f32, tag="rect4", name="rect4",
                                        bufs=int(os.environ.get("KDBG_VOLBUFS", "0")) or None)
                        _DBG.setdefault("rect4", []).append(rect4.tensor.name)
                        for k in range(4):
                            w_k = HK[k]
                            Fsrc = Fk[k]
                            fbase = 0
                            G = fp.tile([128, 99], f32, tag="G", name="G")
                            Gv = G[:].rearrange("p (r i) -> p r i", r=11)
                            xtmp = fp.tile([128, 99], f32, tag="xtmp", name="xtmp")
                            for bi, b_ in enumerate((-1, 0, 1)):
                                fap = Fsrc[:]
                                fin = bass.AP(
                                    tensor=fap.tensor,
                                    offset=fap.offset + (fbase + b_ + 1),
                                    ap=[fap.ap[0], [w_k, 11], [1, 9]])
                                wap = W["x", b_][:].rearrange("p (j q) -> p j q", q=64)[
                                    :, :, k * NT + t].unsqueeze(1).to_broadcast([128, 11, 9])
                                if bi == 0:
                                    nc.vector.tensor_tensor(out=Gv, in0=fin, in1=wap, op=Alu.mult)
                                else:
                                    nc.vector.tensor_tensor(
                                        out=xtmp[:].rearrange("p (r i) -> p r i", r=11),
                                        in0=fin, in1=wap, op=Alu.mult)
                                    nc.vector.tensor_add(G[:], G[:], xtmp[:])
                            rv = rect4[:, k * 81:(k + 1) * 81].rearrange("p (j i) -> p j i", j=9)
                            ytmp = fp.tile([128, 81], f32, tag="ytmp", name="ytmp")
                            for ai, a_ in enumerate((-1, 0, 1)):
                                gin = Gv[:, 1 + a_:10 + a_, :]
                                wap = W["y", a_][:].rearrange("p (j q) -> p j q", q=64)[
                                    :, :, k * NT + t].unsqueeze(2).to_broadcast([128, 9, 9])
                                if ai == 0:
                                    nc.vector.tensor_tensor(out=rv, in0=gin, in1=wap, op=Alu.mult)
                                else:
                                    nc.vector.tensor_tensor(
                                        out=ytmp[:].rearrange("p (j i) -> p j i", j=9),
                                        in0=gin, in1=wap, op=Alu.mult)
                                    nc.vector.tensor_add(
                                        rect4[:, k * 81:(k + 1) * 81],
                                        rect4[:, k * 81:(k + 1) * 81], ytmp[:])

                        if dbg_dump and t == 0:
                            nc.sync.dma_start(out=dbg_rect[:], in_=rect4[:])
                        # ---- diamond extraction: out layout [l, k] ----
                        outt = fp.tile([128, NL * 4], f32, tag="outt", name="outt")
                        r4ap = rect4[:]
                        oap = outt[:]
                        for d in range(9):
                            ln = ROWLEN[d]
                            i0 = abs(d - 4)
                            src = bass.AP(
                                tensor=r4ap.tensor, offset=r4ap.offset + d * 9 + i0,
                                ap=[r4ap.ap[0], [1, ln], [81, 4]])
                            dst = bass.AP(
                                tensor=oap.tensor, offset=oap.offset + ROWOFF[d] * 4,
                                ap=[oap.ap[0], [4, ln], [1, 4]])
                            nc.vector.tensor_copy(out=dst, in_=src)
                        nc.sync.dma_start(out=out[t * 128:(t + 1) * 128, :], in_=outt[:])

    nc.compile()
    return nc


def _get_nc():
    if "nc" not in _CACHE:
        _CACHE["nc"] = _build()
    return _CACHE["nc"]


def _in_maps(feat1, feat2, curr_flow):
    maps = []
    for core in range(NCORES):
        m = dict(_consts(core))
        sl = slice(8 * core, 8 * core + 8)
        m["f1"] = np.ascontiguousarray(feat1[:, :, sl, :], dtype=np.float32)
        m["f2"] = np.ascontiguousarray(feat2, dtype=np.float32)
        m["flow"] = np.ascontiguousarray(curr_flow[:, :, sl, :], dtype=np.float32)
        maps.append(m)
    return maps


def _assemble(outs):
    # per core: [2048, 164] -> [4, 8, 64, 4, 41]; concat cores on i axis
    parts = []
    for o in outs:
        a = o.reshape(B, RI, W8, NL, 4).transpose(0, 1, 2, 4, 3)
        parts.append(a)
    return np.concatenate(parts, axis=1)


def kernel(feat1, feat2, curr_flow):
    from concourse.bass_utils import run_bass_kernel_spmd

    nc = _get_nc()
    res = run_bass_kernel_spmd(nc, _in_maps(feat1, feat2, curr_flow), list(range(NCORES)))
    return _assemble([np.asarray(res.results[i]["out"]) for i in range(NCORES)])


# revision 25
# speedup vs baseline: 1.0049x; 1.0049x over previous
"""RAFT correlation-pyramid lookup kernel for 8 trn2 NeuronCores.

Shard: each core takes all 4 batches x 8 pixel rows (i-axis) of the
64x64 grid.  Per core:
  1. matmul (fp32r) f1^T @ [f2 | pooled-f2-pyramids]  -> corr volumes
  2. ACT drains PSUM -> SBUF fp16, DMA to a DRAM bounce tile
  3. element-granular indirect-DMA gathers per-pixel dynamic windows
  4. DVE reconstructs bilinear lookups as separable 3-tap MACs
Weights / gather indices are computed on-device from the flow input.
"""

import os
import sys

import numpy as np

sys.path.insert(0, "/opt/trn_rl_repo")

B = 4
C = 256
H8 = W8 = 64
RI = 8  # i-rows per core
NCORES = 8
NT = 16  # pixel tiles per core (128 pixels each)
NPIX = NT * 128

HK = [64, 32, 16, 8]  # level map sizes
SEG = [0, 4096, 5120, 5376]  # level offsets within a pixel's volume
VOLW = 5440  # per-pixel volume length (all 4 levels)
L0LEN = 651  # 10*64+11
L123LEN = 331  # 10*32+11  (levels 2,3 padded to this)
NL = 41  # diamond offsets
ROWLEN = [1, 3, 5, 7, 9, 7, 5, 3, 1]  # diamond row lengths (dy=-4..4)
ROWOFF = np.concatenate([[0], np.cumsum(ROWLEN)[:-1]]).tolist()

_CACHE = {}
_DBG = {}


def _consts(core):
    """Constant input tensors for one core (fp32)."""
    p = np.arange(128)
    t = np.arange(NT)
    k = np.arange(4)
    j = np.arange(9)
    hk = np.array(HK, dtype=np.float64)

    # i/j pixel coordinates:  t=(b,ihi), part=(i2,jcol)
    ihi = t % 4
    i2 = p // 64
    iconst = (8 * core + ihi[None, :] * 2 + i2[:, None]).astype(np.float32)
    jconst = (p % 64).astype(np.float32)[:, None]

    cinvs64 = np.broadcast_to((0.5**k)[:, None], (4, NT))
    chi64 = np.broadcast_to((hk + 5.2)[:, None], (4, NT))
    ct64 = np.broadcast_to(((hk - 1.0) / hk)[:, None], (4, NT))
    cw64 = np.broadcast_to(hk[:, None], (4, NT))
    segk = np.array(SEG, dtype=np.float64)
    cbase = p[:, None, None] * VOLW + VOLW + segk[None, :, None] - hk[None, :, None] - 1.0
    cbase64 = np.broadcast_to(cbase, (128, 4, NT))

    def rep64(a):  # [4,NT] -> [128, 64]
        return np.broadcast_to(a[None], (128, 4, NT)).reshape(128, 64).astype(np.float32)

    def rep576(a):  # [9,4] -> [128, 576]
        x = np.broadcast_to(a[:, :, None], (9, 4, NT))
        return np.broadcast_to(x[None], (128, 9, 4, NT)).reshape(128, 576).astype(np.float32)

    cdyt = rep576((j - 4)[:, None] * ((hk - 1.0) / hk)[None, :])
    ch05 = rep576(np.broadcast_to((hk - 0.5)[None, :], (9, 4)))
    ch15 = rep576(np.broadcast_to((hk - 1.5)[None, :], (9, 4)))
    cj = rep576(np.broadcast_to(j[:, None].astype(np.float64), (9, 4)))
    csk = rep576(np.broadcast_to((0.0625 * 0.25**k)[None, :], (9, 4)))

    return {
        "iconst": iconst,
        "jconst": jconst.astype(np.float32),
        "cinvs64": rep64(cinvs64),
        "chi64": rep64(chi64),
        "ct64": rep64(ct64),
        "cw64": rep64(cw64),
        "cbase64": cbase64.reshape(128, 64).astype(np.float32),
        "cdyt": cdyt,
        "ch05": ch05,
        "ch15": ch15,
        "cj": cj,
        "csk": csk,
    }


CSHAPES = {
    "iconst": [128, NT], "jconst": [128, 1],
    "cinvs64": [128, 64], "chi64": [128, 64], "ct64": [128, 64],
    "cw64": [128, 64], "cbase64": [128, 64],
    "cdyt": [128, 576], "ch05": [128, 576], "ch15": [128, 576],
    "cj": [128, 576], "csk": [128, 576],
}


def _build():
    import concourse.bass as bass
    import concourse.tile as tile
    from concourse import bacc, mybir

    f32 = mybir.dt.float32
    f32r = mybir.dt.float32r
    f16 = mybir.dt.float16
    i32 = mybir.dt.int32
    Alu = mybir.AluOpType

    nc = bacc.Bacc("TRN2", target_bir_lowering=False, debug=False, num_devices=NCORES)

    f1 = nc.dram_tensor("f1", [B, C, RI, W8], f32r, kind="ExternalInput")
    f2 = nc.dram_tensor("f2", [B, C, H8, W8], f32r, kind="ExternalInput")
    flow = nc.dram_tensor("flow", [B, 2, RI, W8], f32, kind="ExternalInput")
    cdram = {n: nc.dram_tensor(n, s, f32, kind="ExternalInput") for n, s in CSHAPES.items()}
    out = nc.dram_tensor("out", [NPIX, NL * 4], f32, kind="ExternalOutput")
    dbg_dump = os.environ.get("KDBG_DUMP", "0") == "1"
    if dbg_dump:
        dbg_idx = nc.dram_tensor("dbg_idx", [128, 64], i32, kind="ExternalOutput")
        dbg_w = nc.dram_tensor("dbg_w", [128, 6 * 576], f32, kind="ExternalOutput")
        dbg_vol = nc.dram_tensor("dbg_vol", [128, VOLW], f16, kind="ExternalOutput")
        dbg_f = nc.dram_tensor("dbg_f", [128, 1244], f16, kind="ExternalOutput")
        dbg_rect = nc.dram_tensor("dbg_rect", [128, 324], f32, kind="ExternalOutput")

    with tile.TileContext(nc) as tc:
        with (
            tc.tile_pool(name="const", bufs=1) as cp,
            tc.tile_pool(name="wts", bufs=1) as wp,
        ):
            # ---- flow -> lookup weights + gather indices (DVE) ----
            # (loaded FIRST so the DVE weight chain starts immediately and
            #  overlaps the f1/f2 input loads)
            W = {}
            idx32 = wp.tile([128, 64], i32, tag="idx32", name="idx32")
            with tc.tile_pool(name="wscratch", bufs=1) as sp:
                fy = sp.tile([128, NT], f32, tag="fy", name="fy")
                fx = sp.tile([128, NT], f32, tag="fx", name="fx")
                fsrc = flow[:].rearrange("b c (ihi i2) j -> c (i2 j) b ihi", i2=2)
                for b in range(B):
                    nc.sync.dma_start(out=fy[:, b * 4:(b + 1) * 4], in_=fsrc[0, :, b])
                    nc.sync.dma_start(out=fx[:, b * 4:(b + 1) * 4], in_=fsrc[1, :, b])
                ct = {}
                for n, s in CSHAPES.items():
                    ct[n] = cp.tile(s, f32, tag=n, name=n)
                    nc.sync.dma_start(out=ct[n][:], in_=cdram[n][:])

                yc = sp.tile([128, NT], f32, tag="yc", name="yc")
                xc = sp.tile([128, NT], f32, tag="xc", name="xc")
                nc.vector.tensor_add(yc[:], fy[:], ct["iconst"][:])
                nc.vector.tensor_add(xc[:], fx[:], ct["jconst"][:].to_broadcast([128, NT]))

                Y0J0 = {}
                for ax, coord in (("y", yc), ("x", xc)):
                    ck = sp.tile([128, 64], f32, tag=f"ck_{ax}", name=f"ck{ax}")
                    nc.vector.tensor_tensor(
                        out=ck[:].rearrange("p (k t) -> p k t", k=4),
                        in0=coord[:].unsqueeze(1).to_broadcast([128, 4, NT]),
                        in1=ct["cinvs64"][:].rearrange("p (k t) -> p k t", k=4), op=Alu.mult)
                    nc.vector.tensor_tensor(out=ck[:], in0=ck[:], in1=ct["chi64"][:], op=Alu.min)
                    nc.vector.tensor_scalar_max(ck[:], ck[:], -5.2)
                    nc.vector.tensor_tensor(out=ck[:], in0=ck[:], in1=ct["ct64"][:], op=Alu.mult)

                    ybar = sp.tile([128, 576], f32, tag=f"ybar_{ax}", name=f"ybar{ax}")
                    nc.vector.tensor_tensor(
                        out=ybar[:].rearrange("p (j q) -> p j q", j=9),
                        in0=ck[:].unsqueeze(1).to_broadcast([128, 9, 64]),
                        in1=ct["cdyt"][:].rearrange("p (j q) -> p j q", j=9), op=Alu.add)
                    # exact floor: round-to-nearest via +2^23, then correct
                    frac = sp.tile([128, 576], f32, tag=f"frac_{ax}", name=f"frac{ax}")
                    y0f = sp.tile([128, 576], f32, tag=f"y0f_{ax}", name=f"y0f{ax}")
                    cmp = sp.tile([128, 576], f32, tag=f"cmp_{ax}", name=f"cmp{ax}")
                    nc.vector.tensor_scalar_add(y0f[:], ybar[:], 12582912.0)
                    nc.vector.tensor_scalar_add(y0f[:], y0f[:], -12582912.0)
                    nc.vector.tensor_tensor(out=cmp[:], in0=y0f[:], in1=ybar[:], op=Alu.is_gt)
                    nc.vector.tensor_sub(y0f[:], y0f[:], cmp[:])
                    nc.vector.tensor_sub(frac[:], ybar[:], y0f[:])
                    Y0J0[ax] = y0f

                    v0 = sp.tile([128, 576], f32, tag=f"v0_{ax}", name=f"v0{ax}")
                    v1 = sp.tile([128, 576], f32, tag=f"v1_{ax}", name=f"v1{ax}")
                    tmp = sp.tile([128, 576], f32, tag=f"tmp_{ax}", name=f"tmp{ax}")
                    # valid(y0):   y0 in [0, h-1]
                    nc.vector.tensor_scalar(v0[:], y0f[:], -0.1, None, op0=Alu.is_ge)
                    nc.vector.tensor_tensor(out=tmp[:], in0=y0f[:], in1=ct["ch05"][:], op=Alu.is_le)
                    nc.vector.tensor_mul(v0[:], v0[:], tmp[:])
                    # valid(y0+1): y0 in [-1, h-2]
                    nc.vector.tensor_scalar(v1[:], y0f[:], -1.1, None, op0=Alu.is_ge)
                    nc.vector.tensor_tensor(out=tmp[:], in0=y0f[:], in1=ct["ch15"][:], op=Alu.is_le)
                    nc.vector.tensor_mul(v1[:], v1[:], tmp[:])

                    w0 = sp.tile([128, 576], f32, tag=f"w0_{ax}", name=f"w0{ax}")
                    w1 = sp.tile([128, 576], f32, tag=f"w1_{ax}", name=f"w1{ax}")
                    nc.vector.tensor_mul(w1[:], frac[:], v1[:])
                    nc.vector.tensor_scalar(w0[:], frac[:], 1.0, -1.0, op0=Alu.subtract, op1=Alu.mult)
                    nc.vector.tensor_mul(w0[:], w0[:], v0[:])
                    if ax == "x":  # fold level scale (1/16 * 4^-k) into x weights
                        nc.vector.tensor_mul(w0[:], w0[:], ct["csk"][:])
                        nc.vector.tensor_mul(w1[:], w1[:], ct["csk"][:])

                    ey = sp.tile([128, 576], f32, tag=f"ey_{ax}", name=f"ey{ax}")
                    nc.vector.tensor_tensor(
                        out=ey[:].rearrange("p (j q) -> p j q", j=9),
                        in0=y0f[:, 0:64].unsqueeze(1).to_broadcast([128, 9, 64]),
                        in1=y0f[:].rearrange("p (j q) -> p j q", j=9), op=Alu.subtract)
                    nc.vector.tensor_add(ey[:], ey[:], ct["cj"][:])

                    # single weight tile per axis: b-planes (-1,0,1) at 0/576/1152
                    Wt = wp.tile([128, 3 * 576], f32, tag=f"W_{ax}", name=f"W{ax}")
                    wm1, w_0, wp1 = Wt[:, 0:576], Wt[:, 576:1152], Wt[:, 1152:1728]
                    tmp2 = sp.tile([128, 576], f32, tag=f"tmp2_{ax}", name=f"tmp2{ax}")
                    nc.vector.tensor_mul(wm1, w0[:], ey[:])
                    nc.vector.tensor_mul(tmp2[:], w1[:], ey[:])
                    nc.vector.tensor_sub(w_0, w0[:], wm1)
                    nc.vector.tensor_add(w_0, w_0, tmp2[:])
                    nc.vector.tensor_sub(wp1, w1[:], tmp2[:])
                    W[ax] = Wt

                # gather indices: [128, 64] int32 (fp16-element offsets)
                idxf = sp.tile([128, 64], f32, tag="idxf", name="idxf")
                nc.vector.tensor_tensor(out=idxf[:], in0=Y0J0["y"][:, 0:64], in1=ct["cw64"][:], op=Alu.mult)
                nc.vector.tensor_add(idxf[:], idxf[:], Y0J0["x"][:, 0:64])
                nc.vector.tensor_add(idxf[:], idxf[:], ct["cbase64"][:])
                # store as [t, k] so per-tile index slices are contiguous
                nc.vector.tensor_copy(
                    out=idx32[:].rearrange("p (t k) -> p t k", k=4).transpose([0, 2, 1]),
                    in_=idxf[:].rearrange("p (k t) -> p k t", t=NT))

            if dbg_dump:
                nc.sync.dma_start(out=dbg_idx[:], in_=idx32[:])
                for qi, (ax, bq) in enumerate((("x", 0), ("x", 1), ("x", 2), ("y", 0), ("y", 1), ("y", 2))):
                    nc.sync.dma_start(out=dbg_w[:, qi * 576:(qi + 1) * 576],
                                      in_=W[ax][:, bq * 576:(bq + 1) * 576])

            # f1 lhsT tiles: [128c, 512pix] per (b, kc)
            f1t = {}
            for b in range(B):
                for kc in range(2):
                    f1t[b, kc] = cp.tile([128, 512], f32r, tag=f"f1_{b}_{kc}", name=f"f1t{b}{kc}")
                    src = f1[b, kc * 128:(kc + 1) * 128, :, :].rearrange("c i j -> c (i j)")
                    nc.sync.dma_start(out=f1t[b, kc][:], in_=src)

            # zero row for DRAM pad rows
            zrow = cp.tile([1, VOLW], f16, tag="zrow", name="zrow")
            nc.vector.memset(zrow[:], 0.0)

            # matmul N-chunk plan: (source, src_off, width, vol_off)
            # L2+L3 share one PSUM tile ([0:256]=L2, [256:320]=L3)
            nplan = [("f2", n * 512, 512, n * 512) for n in range(8)] + [
                ("pyr", 0, 512, 4096), ("pyr", 512, 512, 4608),
                ("pyr", 1024, 320, 5120)]

            with (
                tc.tile_pool(name="main", bufs=2) as mp,
                tc.tile_pool(name="fio", bufs=3) as fp,
                tc.tile_pool(name="dram", bufs=3, space="DRAM") as dp,
                tc.tile_pool(name="psum", bufs=8, space="PSUM") as pp,
            ):
                nb = int(os.environ.get("KDBG_NB", str(B)))
                nm_ = int(os.environ.get("KDBG_NM", "4"))
                for b in range(nb):
                    f2t = {}
                    pyrt = {}
                    for kc in range(2):
                        f2t[kc] = mp.tile([128, 4096], f32r, tag=f"f2_{kc}", name=f"f2t{kc}")
                        src = f2[b, kc * 128:(kc + 1) * 128, :, :].rearrange("c u v -> c (u v)")
                        nc.sync.dma_start(out=f2t[kc][:], in_=src)
                        # pooled pyramids (pure sums; scales folded into x-weights)
                        pyrt[kc] = mp.tile([128, 1344], f32r, tag=f"pyr_{kc}", name=f"pyrt{kc}")
                        A = mp.tile([128, 2048], f32r, tag="poolA", name="poolA")
                        s3 = f2t[kc][:].rearrange("p (x two) -> p x two", two=2)
                        nc.vector.tensor_tensor(out=A[:], in0=s3[:, :, 0], in1=s3[:, :, 1], op=Alu.add)
                        a3 = A[:].rearrange("p (uh two v) -> p uh two v", two=2, v=32)
                        nc.vector.tensor_tensor(
                            out=pyrt[kc][:, 0:1024].rearrange("p (u v) -> p u v", v=32),
                            in0=a3[:, :, 0, :], in1=a3[:, :, 1, :], op=Alu.add)
                        s3 = pyrt[kc][:, 0:1024].rearrange("p (x two) -> p x two", two=2)
                        nc.vector.tensor_tensor(out=A[:, 0:512], in0=s3[:, :, 0], in1=s3[:, :, 1], op=Alu.add)
                        a3 = A[:, 0:512].rearrange("p (uh two v) -> p uh two v", two=2, v=16)
                        nc.vector.tensor_tensor(
                            out=pyrt[kc][:, 1024:1280].rearrange("p (u v) -> p u v", v=16),
                            in0=a3[:, :, 0, :], in1=a3[:, :, 1, :], op=Alu.add)
                        s3 = pyrt[kc][:, 1024:1280].rearrange("p (x two) -> p x two", two=2)
                        nc.vector.tensor_tensor(out=A[:, 0:128], in0=s3[:, :, 0], in1=s3[:, :, 1], op=Alu.add)
                        a3 = A[:, 0:128].rearrange("p (uh two v) -> p uh two v", two=2, v=8)
                        nc.vector.tensor_tensor(
                            out=pyrt[kc][:, 1280:1344].rearrange("p (u v) -> p u v", v=8),
                            in0=a3[:, :, 0, :], in1=a3[:, :, 1, :], op=Alu.add)

                    for m in range(nm_):
                        t = b * 4 + m
                        volsb = fp.tile([128, VOLW], f16, tag="volsb", name="volsb", bufs=2)
                        for (srcname, soff, width, doff) in nplan:
                            ps = pp.tile([128, width], f32, tag="ps", name="ps")
                            rhs_t = f2t if srcname == "f2" else pyrt
                            for kc in range(2):
                                nc.tensor.matmul(
                                    out=ps[:],
                                    lhsT=f1t[b, kc][:, m * 128:(m + 1) * 128],
                                    rhs=rhs_t[kc][:, soff:soff + width],
                                    start=(kc == 0), stop=(kc == 1))
                            nc.scalar.copy(out=volsb[:, doff:doff + width], in_=ps[:])

                        vols = dp.tile([130, VOLW], f16, tag="vols", name="vols",
                                       bufs=int(os.environ.get("KDBG_VOLBUFS", "0")) or None)
                        _DBG.setdefault("vols", []).append(vols.tensor.name)
                        nc.sync.dma_start(out=vols[0:1, :], in_=zrow[:])
                        nc.sync.dma_start(out=vols[129:130, :], in_=zrow[:])
                        nc.sync.dma_start(out=vols[1:129, :], in_=volsb[:])

                        vflat = vols[:].rearrange("a b -> (a b)").unsqueeze(1)
                        Fk = []
                        for k in range(4):
                            flen = 10 * HK[k] + 11
                            ftile = fp.tile([128, flen], f16, tag=f"F{k}", name=f"F{k}")
                            nc.gpsimd.indirect_dma_start(
                                out=ftile[:], out_offset=None, in_=vflat,
                                in_offset=bass.IndirectOffsetOnAxis(
                                    ap=idx32[:, t * 4 + k:t * 4 + k + 1], axis=0))
                            Fk.append(ftile)
                        _DBG.setdefault("F0", []).append(Fk[0].tensor.name)
                        if dbg_dump and t == 0:
                            nc.sync.dma_start(out=dbg_vol[:], in_=volsb[:])
                            fo = 0
                            for k in range(4):
                                fl = 10 * HK[k] + 11
                                nc.sync.dma_start(out=dbg_f[:, fo:fo + fl], in_=Fk[k][:])
                                fo += fl

                        # ---- separable 3-tap reconstruction (fused mult+reduce) ----
                        rect4 = fp.tile([128, 324], f32, tag="rect4", name="rect4",
                                        bufs=int(os.environ.get("KDBG_VOLBUFS", "0")) or None)
                        _DBG.setdefault("rect4", []).append(rect4.tensor.name)
                        for k in range(4):
                            w_k = HK[k]
                            fap = Fk[k][:]
                            kt = k * NT + t
                            # stage X: tmpx[r,i,b] = F[r*w + i + b] * WX[b, j=i]
                            tmpx = fp.tile([128, 297], f32, tag="tmpx", name="tmpx")
                            fin = bass.AP(tensor=fap.tensor, offset=fap.offset,
                                          ap=[fap.ap[0], [w_k, 11], [1, 9], [1, 3]])
                            wxap = bass.AP(tensor=W["x"][:].tensor,
                                           offset=W["x"][:].offset + kt,
                                           ap=[W["x"][:].ap[0], [0, 11], [64, 9], [576, 3]])
                            txv = bass.AP(tensor=tmpx[:].tensor, offset=tmpx[:].offset,
                                          ap=[tmpx[:].ap[0], [27, 11], [3, 9], [1, 3]])
                            nc.vector.tensor_tensor(out=txv, in0=fin, in1=wxap, op=Alu.mult)
                            G = fp.tile([128, 99], f32, tag="G", name="G")
                            nc.vector.tensor_reduce(
                                out=G[:], in_=tmpx[:].rearrange("p (q b) -> p q b", b=3),
                                axis=mybir.AxisListType.X, op=Alu.add)
                            # stage Y: tmpy[j,i,a] = G[(j+1+a-1)*9 + i] * WY[a, j]
                            tmpy = fp.tile([128, 243], f32, tag="tmpy", name="tmpy")
                            gin = bass.AP(tensor=G[:].tensor, offset=G[:].offset,
                                          ap=[G[:].ap[0], [9, 9], [1, 9], [9, 3]])
                            wyap = bass.AP(tensor=W["y"][:].tensor,
                                           offset=W["y"][:].offset + kt,
                                           ap=[W["y"][:].ap[0], [64, 9], [0, 9], [576, 3]])
                            tyv = bass.AP(tensor=tmpy[:].tensor, offset=tmpy[:].offset,
                                          ap=[tmpy[:].ap[0], [27, 9], [3, 9], [1, 3]])
                            nc.vector.tensor_tensor(out=tyv, in0=gin, in1=wyap, op=Alu.mult)
                            nc.vector.tensor_reduce(
                                out=rect4[:, k * 81:(k + 1) * 81],
                                in_=tmpy[:].rearrange("p (q a) -> p q a", a=3),
                                axis=mybir.AxisListType.X, op=Alu.add)

                        if dbg_dump and t == 0:
                            nc.sync.dma_start(out=dbg_rect[:], in_=rect4[:])
                        # ---- diamond extraction: out layout [l, k] ----
                        outt = fp.tile([128, NL * 4], f32, tag="outt", name="outt")
                        r4ap = rect4[:]
                        oap = outt[:]
                        for d in range(9):
                            ln = ROWLEN[d]
                            i0 = abs(d - 4)
                            src = bass.AP(
                                tensor=r4ap.tensor, offset=r4ap.offset + d * 9 + i0,
                                ap=[r4ap.ap[0], [1, ln], [81, 4]])
                            dst = bass.AP(
                                tensor=oap.tensor, offset=oap.offset + ROWOFF[d] * 4,
                                ap=[oap.ap[0], [4, ln], [1, 4]])
                            nc.vector.tensor_copy(out=dst, in_=src)
                        nc.sync.dma_start(out=out[t * 128:(t + 1) * 128, :], in_=outt[:])

    nc.compile()
    return nc


def _get_nc():
    if "nc" not in _CACHE:
        _CACHE["nc"] = _build()
    return _CACHE["nc"]


def _in_maps(feat1, feat2, curr_flow):
    maps = []
    for core in range(NCORES):
        m = dict(_consts(core))
        sl = slice(8 * core, 8 * core + 8)
        m["f1"] = np.ascontiguousarray(feat1[:, :, sl, :], dtype=np.float32)
        m["f2"] = np.ascontiguousarray(feat2, dtype=np.float32)
        m["flow"] = np.ascontiguousarray(curr_flow[:, :, sl, :], dtype=np.float32)
        maps.append(m)
    return maps


def _assemble(outs):
    # per core: [2048, 164] -> [4, 8, 64, 4, 41]; concat cores on i axis
    parts = []
    for o in outs:
        a = o.reshape(B, RI, W8, NL, 4).transpose(0, 1, 2, 4, 3)
        parts.append(a)
    return np.concatenate(parts, axis=1)


def kernel(feat1, feat2, curr_flow):
    from concourse.bass_utils import run_bass_kernel_spmd

    nc = _get_nc()
    res = run_bass_kernel_spmd(nc, _in_maps(feat1, feat2, curr_flow), list(range(NCORES)))
    return _assemble([np.asarray(res.results[i]["out"]) for i in range(NCORES)])
